# revision 15
# baseline (speedup 1.0000x reference)
"""Causal self-attention (B=4, T=2048, C=1024, H=16) on 8 NeuronCores.

Sharding: data-parallel over batch (4) x tensor-parallel over heads (2 groups
of 8 heads) = 8 cores. Each core computes QKV for its 8 heads, causal
flash-style attention, and a partial output projection (row-parallel).
Host sums the two partial projections per batch and adds b_proj.

All matmul operands are stored fp16 (the PE multiplies at ~fp22 internally,
so fp16's 11-bit mantissa matches fp32r precision while halving memory and
enabling hardware DMA-transpose + fast weight loads). All accumulation is
fp32 in PSUM.

Per-core device kernel (Bass/Tile):
  phase 1: x^T loaded via hardware DMA-transpose (fp16); q^T,k^T [ch,T] and
           v [T,ch] (65-col blocks with a ones column that makes the PV
           matmul emit softmax denominators) via fp16 matmuls; qk bias fused
           into the PSUM->SBUF copy, v bias fused into the v copy (DVE add).
  phase 2: per (head, 512-wide tq chunk): scores^T = k^T.T @ q^T in PSUM,
           exp on ACT (scale=1/8) -> P^T fp16, causal handled by restricting
           diagonal-block columns + affine_select zero-fill, PV accumulate
           y^T[65,512] where row 64 = softmax denominator l. Normalization:
           r=1/l broadcast to [64,512] with a K=1 PE matmul, applied on DVE.
           Only phase-1 chunk 0 runs up front; chunks 1-3 are interleaved
           into the attention stream on a quota schedule (chunk c+1 finishes
           during attention chunk c) so ACT-bound exp work overlaps PE-bound
           qkv matmuls from the very start.
  phase 3 (interleaved per tq chunk): out = y^T.T @ w_proj, DMA out (fp32).

PSUM budget (8 banks): scores ring "sg" 2x[128,1024] = 4 banks, matmul ring
"mm" (qk/v/proj/R) 2x[128,512] = 2 banks, y ring "psy" 2x[65,512] = 2 banks.
Scores get their own ring so interleaved phase-1/proj matmuls never wait on
the exp to free a PSUM buffer.
"""

from contextlib import nullcontext

import numpy as np

import concourse.bass as bass
import concourse.mybir as mybir
from concourse import bacc
from concourse.tile import TileContext
from concourse.bass_utils import run_bass_kernel_spmd

B, T, C, H, D = 4, 2048, 1024, 16, 64
CQ = 512          # q (or k or v) channels per core = 8 heads * 64
HPC = 8           # heads per core
F32 = mybir.dt.float32
F16 = mybir.dt.float16
Exp = mybir.ActivationFunctionType.Exp
is_ge = mybir.AluOpType.is_ge

TCH = 512         # phase-1 T-chunk
NCH = T // TCH    # 4 chunks
VSTR = HPC * (D + 1)   # 520: v_ext per-T-block stride (8 heads x 65)


def build_nc(loop_n=1):
    """loop_n > 1 wraps the whole kernel in a device-side repeat loop
    (benchmarking only -- output is identical every iteration)."""
    nc = bacc.Bacc("TRN2", target_bir_lowering=False, debug=False, num_devices=8)

    x = nc.dram_tensor("x", [T, C], F16, kind="ExternalInput")
    w_qk = nc.dram_tensor("w_qk", [C, 2 * CQ], F16, kind="ExternalInput")
    w_v = nc.dram_tensor("w_v", [C, CQ], F16, kind="ExternalInput")
    b_qk = nc.dram_tensor("b_qk", [1, 2 * CQ], F32, kind="ExternalInput")
    b_vz = nc.dram_tensor("b_vz", [128, CQ], F16, kind="ExternalInput")
    onesz = nc.dram_tensor("onesz", [128, 128], F16, kind="ExternalInput")
    w_pj = nc.dram_tensor("w_pj", [CQ, C], F16, kind="ExternalInput")
    out = nc.dram_tensor("out", [T, C], F16, kind="ExternalOutput")

    with TileContext(nc) as tc:
        with (
            tc.tile_pool(name="const", bufs=1) as pc,
            tc.tile_pool(name="persist", bufs=1) as pp,
            tc.tile_pool(name="work", bufs=2) as pw,
            tc.tile_pool(name="psum", bufs=2, space="PSUM") as ps,
            tc.For_i(0, loop_n, 1, staggered_reset=True)
            if loop_n > 1 else nullcontext(),
        ):
            # ---- persistent activations ----
            qT = [pp.tile([128, T], F16, name=f"qT{m}") for m in range(4)]
            # per-head k^T, zero-padded to K=128 so the scores matmul streams
            # the full qT tile at full SBUF bandwidth (the zero half
            # multiplies the sibling head's rows away)
            kZ = [pp.tile([128, T], F16, name=f"kZ{i}") for i in range(HPC)]
            for i in range(HPC):
                z0 = 64 * (1 - i % 2)
                nc.vector.memset(kZ[i][z0:z0 + 64, :], 0.0)
            yT = [pp.tile([128, T], F16, name=f"yT{m}") for m in range(4)]
            v_ext = pp.tile([128, (T // 128) * VSTR], F16, name="v_ext")
            v_ones = v_ext[:].rearrange(
                "p (t i d) -> p t i d", i=HPC, d=D + 1
            )[:, :, :, D:D + 1]
            nc.gpsimd.memset(v_ones, 1.0)

            # ---- constants (emission order = DMA issue order; x chunk-0
            # transposes are issued first inside phase1 below) ----
            w_qk_sb = pc.tile([128, 8 * 1024], F16, name="w_qk_sb")
            b_qk2 = pc.tile([128, 8], F32, name="b_qk2")
            b_vz_sb = pc.tile([128, CQ], F16, name="b_vz_sb")
            onesz_sb = pc.tile([128, 128], F16, name="onesz_sb")
            w_v_sb = pc.tile([128, 8 * 512], F16, name="w_v_sb")
            w_pj_sb = pc.tile([128, 4 * 1024], F16, name="w_pj_sb")

            def load_consts_early():
                for kk in range(8):
                    nc.sync.dma_start(
                        out=w_qk_sb[:, kk * 1024:(kk + 1) * 1024],
                        in_=w_qk[kk * 128:(kk + 1) * 128, :],
                    )
                # per-channel qk bias as [128, 8] (partition = ch within tile)
                nc.sync.dma_start(
                    out=b_qk2[:], in_=b_qk[0, :].rearrange("(m p) -> p m", p=128)
                )

            def load_consts_mid():
                nc.sync.dma_start(out=onesz_sb[:], in_=onesz[:])
                nc.sync.dma_start(out=b_vz_sb[:], in_=b_vz[:])
                for kk in range(8):
                    nc.sync.dma_start(
                        out=w_v_sb[:, kk * 512:(kk + 1) * 512],
                        in_=w_v[kk * 128:(kk + 1) * 128, :],
                    )

            def load_consts_late():
                for kk in range(4):
                    nc.sync.dma_start(
                        out=w_pj_sb[:, kk * 1024:(kk + 1) * 1024],
                        in_=w_pj[kk * 128:(kk + 1) * 128, :],
                    )

            def phase1_chunk_steps(ct):
                """Yield emission closures for one phase-1 chunk, so chunks
                can be interleaved into the attention stream."""
                T0 = ct * TCH
                xT_c = pw.tile([128, 8 * TCH], F16, name="xT_c", tag="xT_c", bufs=2)

                def xload():
                    # x^T tiles straight from DRAM via hardware DMA transpose
                    for kk in range(8):
                        nc.sync.dma_start_transpose(
                            xT_c[:, kk * TCH:(kk + 1) * TCH],
                            x[T0:T0 + TCH, kk * 128:(kk + 1) * 128],
                        )

                def qk1(m):
                    # q,k: out^T layout [ch, T-chunk]; bias fused into copy
                    qk_ps = ps.tile([128, TCH], F32, name="qk_ps", tag="mm", bufs=2)
                    for kk in range(8):
                        nc.tensor.matmul(
                            qk_ps[:],
                            w_qk_sb[:, kk * 1024 + m * 128:kk * 1024 + (m + 1) * 128],
                            xT_c[:, kk * TCH:(kk + 1) * TCH],
                            start=(kk == 0),
                            stop=(kk == 7),
                        )
                    if m < 4:
                        nc.vector.tensor_scalar_add(
                            qT[m][:, T0:T0 + TCH], qk_ps[:], b_qk2[:, m:m + 1]
                        )
                    else:
                        for half in range(2):
                            ih = 2 * (m - 4) + half
                            rows = slice(64 * half, 64 * half + 64)
                            nc.vector.tensor_scalar_add(
                                kZ[ih][rows, T0:T0 + TCH],
                                qk_ps[rows, :],
                                b_qk2[rows, m:m + 1],
                            )

                def vpart(tt):
                    # v: natural layout [T-block, ch], interleaved into v_ext;
                    # bias added on DVE during the copy (b_vz rows replicated)
                    v_ps = ps.tile([128, CQ], F32, name="v_ps", tag="mm", bufs=2)
                    for kk in range(8):
                        nc.tensor.matmul(
                            v_ps[:],
                            xT_c[:, kk * TCH + tt * 128:kk * TCH + (tt + 1) * 128],
                            w_v_sb[:, kk * 512:(kk + 1) * 512],
                            start=(kk == 0),
                            stop=(kk == 7),
                        )
                    tb = ct * (TCH // 128) + tt
                    dst = v_ext[:, tb * VSTR:(tb + 1) * VSTR].rearrange(
                        "p (i d) -> p i d", d=D + 1
                    )[:, :, 0:D]
                    src = v_ps[:].rearrange("p (i d) -> p i d", d=D)
                    bias = b_vz_sb[:].rearrange("p (i d) -> p i d", d=D)
                    nc.vector.tensor_add(dst, src, bias)

                yield xload
                for m in range(8):
                    yield lambda m=m: qk1(m)
                for t0 in range(TCH // 128):
                    yield lambda t0=t0: vpart(t0)

            # PV pipeline state carried ACROSS heads: each entry is one
            # scores+exp group whose PV matmuls haven't been emitted yet.
            # Keeping the diagonal groups of head i pending into head i+1's
            # stream means the in-order PE never waits on the exp/select
            # chain (it runs head i+1's scores matmuls meanwhile). Entries
            # are tagged with a head serial so normalize() can drain exactly
            # the groups of the head it is about to read.
            pending = []  # [(serial, P tile, y_ps, vslice, nblk, items)]
            head_serial = [0]

            def _emit_pv(entry):
                _, P, y_ps, vslice, nblk, items = entry
                for tkb, oc0, pc0, w in items:
                    nc.tensor.matmul(
                        y_ps[:, oc0:oc0 + w],
                        vslice(tkb),
                        P[:, pc0:pc0 + w],
                        start=(tkb == 0),
                        stop=(tkb == nblk - 1),
                        skip_group_check=True,
                    )

            def flush(depth=0):
                while len(pending) > depth:
                    _emit_pv(pending.pop(0))

            def flush_head(serial):
                while pending and pending[0][0] <= serial:
                    _emit_pv(pending.pop(0))

            def attention(c, i, filler=None):
                """Emit scores+exp groups; PV runs 2 groups behind through
                the shared `pending` pipeline. Returns (y_ps, m, p0, c) for
                deferred normalization."""
                m = i // 2
                p0 = 64 * (i % 2)
                nblk = 4 * c + 4
                head_serial[0] += 1
                serial = head_serial[0]
                y_ps = ps.tile([D + 1, 512], F32, name="y_ps", tag="psy", bufs=2)

                def vslice(tkb):
                    return v_ext[
                        :, tkb * VSTR + i * (D + 1):tkb * VSTR + (i + 1) * (D + 1)
                    ]

                def group(items):
                    """One psum tile + one exp over several blocks.
                    items: (tkb, out_col0, p_col0, width, straddler)."""
                    total = items[-1][2] + items[-1][3]
                    s_g = ps.tile([128, 1024], F32, name="s_g", tag="sg", bufs=2)
                    P_g = pw.tile([128, 1024], F16, name="P_g", tag="P_t", bufs=8)
                    for tkb, oc0, pc0, w, _ in items:
                        nc.tensor.matmul(
                            s_g[:, pc0:pc0 + w],
                            kZ[i][:, tkb * 128:(tkb + 1) * 128],
                            qT[m][:, c * 512 + oc0:(c + 1) * 512],
                            start=True,
                            stop=True,
                        )
                    nc.scalar.activation(
                        P_g[:, 0:total], s_g[:, 0:total], Exp, scale=0.125)
                    for tkb, oc0, pc0, w, straddler in items:
                        if straddler:
                            # keep where (piece-local y) >= x
                            nc.gpsimd.affine_select(
                                out=P_g[:, pc0:pc0 + w],
                                in_=P_g[:, pc0:pc0 + w],
                                compare_op=is_ge,
                                fill=0.0,
                                base=0,
                                pattern=[[1, w]],
                                channel_multiplier=-1,
                            )
                    flush(depth=3)
                    pending.append(
                        (serial, P_g, y_ps, vslice, nblk,
                         [it[:4] for it in items]))
                    if filler is not None:
                        filler()

                # full (below-diagonal) blocks in pairs; diagonal straddlers
                # packed j0+j1 and j2+j3 to amortize ACT fixed cost
                for pair in range(2 * c):
                    group([(2 * pair, 0, 0, 512, False),
                           (2 * pair + 1, 0, 512, 512, False)])
                group([(4 * c, 0, 0, 512, True),
                       (4 * c + 1, 128, 512, 384, True)])
                group([(4 * c + 2, 256, 0, 256, True),
                       (4 * c + 3, 384, 256, 128, True)])
                return (y_ps, m, p0, c, serial)

            o_tiles = {}
            r2_tiles = {}

            def normalize_a(pend):
                """Head i+1's slot: drain head i's PVs, stash unnormalized
                y^T into yT (fp16) and 1/l into the pair's r2 row. Frees the
                y_ps PSUM bank; involves no PE work at all."""
                y_ps, m, p0, c, serial = pend
                flush_head(serial)
                with nc.allow_low_precision(reason="fp16 partial y"):
                    nc.vector.tensor_copy(
                        yT[m][p0:p0 + 64, c * 512:(c + 1) * 512], y_ps[0:D, :])
                r2 = r2_tiles.setdefault(
                    m, pw.tile([65, 512], F16, name="r2", tag="r2", bufs=2))
                with nc.allow_low_precision(reason="fp16 reciprocal"):
                    nc.vector.reciprocal(r2[p0:p0 + 1, :], y_ps[D:D + 1, :])

            def normalize_b(m, c):
                """Per head pair: broadcast both heads' 1/l rows across their
                64-partition halves with two K=1 matmuls into one PSUM bank,
                then one in-place [128,512] mul normalizes both heads.
                Emitted >=1 slot after the pair's reciprocals so the PE never
                waits on the DVE."""
                r2 = r2_tiles.pop(m)
                ones1 = onesz_sb[0:1, 0:64]
                R_ps = ps.tile([128, 512], F32, name="R_ps", tag="mm", bufs=2)
                nc.tensor.matmul(
                    R_ps[0:64, :], ones1, r2[0:1, :], start=True, stop=True)
                nc.tensor.matmul(
                    R_ps[64:128, :], onesz_sb[64:65, 0:64], r2[64:65, :],
                    start=True, stop=True)
                R_sb = pw.tile([128, 512], F16, name="R_sb", tag="R_sb", bufs=2)
                with nc.allow_low_precision(reason="fp16 reciprocal"):
                    nc.vector.tensor_copy(R_sb[:], R_ps[:])
                cols = slice(c * 512, (c + 1) * 512)
                with nc.allow_low_precision(reason="fp16 matches PE fp22 input precision"):
                    nc.vector.tensor_mul(yT[m][:, cols], yT[m][:, cols], R_sb[:])

            def proj(mt, kks=(0, 1, 2, 3), finish=True, nns=(0, 1)):
                """Partial-k projection: kks selects which yT k-blocks to
                accumulate this call. finish=False stashes the partial in o_t
                (fp16) to be completed later by a second call -- used for the
                last chunk so most of its projection hides inside the
                ACT-bound end of attention."""
                o_t = o_tiles.setdefault(
                    mt, pw.tile([128, C], F16, name="o_t", tag="o_t", bufs=6))
                for nn in nns:
                    pj_ps = ps.tile([128, 512], F32, name="pj_ps", tag="mm", bufs=2)
                    for kk in kks:
                        nc.tensor.matmul(
                            pj_ps[:],
                            yT[kk][:, mt * 128:(mt + 1) * 128],
                            w_pj_sb[:, kk * 1024 + nn * 512:kk * 1024 + (nn + 1) * 512],
                            start=(kk == kks[0]),
                            stop=(kk == kks[-1]),
                        )
                    dst = o_t[:, nn * 512:(nn + 1) * 512]
                    with nc.allow_low_precision(reason="fp16 partial output"):
                        if kks[0] == 0:
                            nc.vector.tensor_copy(dst, pj_ps[:])
                        else:
                            nc.vector.tensor_add(dst, dst, pj_ps[:])
                if finish:
                    nc.gpsimd.dma_start(
                        out=out[mt * 128:(mt + 1) * 128, :], in_=o_t[:])
                    del o_tiles[mt]

            # ---- emission: x chunk-0 transposes first (the critical DMA),
            # then the weights they meet in the first matmuls; only chunk 0
            # is computed up front. Later chunks' phase-1 and earlier chunks'
            # projections are spread INSIDE the attention stream (one small
            # step after individual score groups) so the in-order PE FIFO
            # always holds non-dependent work when a score group waits on
            # the exp/select chain.
            c0_steps = list(phase1_chunk_steps(0))
            c0_steps[0]()          # xload chunk 0
            load_consts_early()    # w_qk, b_qk2
            load_consts_mid()      # onesz, b_vz, w_v
            for step in c0_steps[1:]:
                step()
            load_consts_late()     # w_pj

            pend = None
            proj_q = []            # proj half-steps (nn granularity)
            proj_quota = {0: 0, 1: 4, 2: 8, 3: 12}   # in halves
            last_c = T // 512 - 1
            for c in range(T // 512):
                p1_items = (list(phase1_chunk_steps(c + 1))
                            if c + 1 < NCH else [])
                quota = min(proj_quota[c], len(proj_q))
                total_groups = HPC * (2 * c + 2)
                p1_deadline = total_groups - 8   # p1 done one head early
                state = {"g": 0, "p1": 0, "pj": 0}

                def filler(p1_items=p1_items, quota=quota,
                           total_groups=total_groups,
                           p1_deadline=p1_deadline, state=state):
                    state["g"] += 1
                    g = state["g"]
                    while (state["p1"] < len(p1_items)
                           and state["p1"] * p1_deadline < len(p1_items) * g):
                        p1_items[state["p1"]]()
                        state["p1"] += 1
                    while (state["pj"] < quota
                           and state["pj"] * total_groups < quota * g):
                        proj_q.pop(0)()
                        state["pj"] += 1

                for i in range(HPC):
                    nxt = attention(c, i, filler)
                    if pend is not None:
                        normalize_a(pend)
                        if pend[2] == 64:   # odd head: its pair is complete
                            normalize_b(pend[1], pend[3])
                    pend = nxt
                    # last chunk: its own projection (k-blocks 0-2, i.e.
                    # heads 0-5, already normalized) fills the ACT-bound end
                    if c == last_c and i >= 6:
                        for mt in (4 * c + 2 * (i - 6), 4 * c + 2 * (i - 6) + 1):
                            proj(mt, kks=(0, 1, 2), finish=False)
                # drain any unconsumed fillers, then close the chunk
                for f in p1_items[state["p1"]:]:
                    f()
                for _ in range(state["pj"], quota):
                    proj_q.pop(0)()
                flush()
                normalize_a(pend)
                normalize_b(pend[1], pend[3])
                pend = None
                if c != last_c:
                    for mt in range(4 * c, 4 * c + 4):
                        proj_q.append(
                            lambda mt=mt: proj(mt, nns=(0,), finish=False))
                        proj_q.append(
                            lambda mt=mt: proj(mt, nns=(1,), finish=True))
            for f in proj_q:
                f()
            for mt in range(4 * last_c, 4 * last_c + 4):
                proj(mt, kks=(3,), finish=True)

    nc.compile()
    return nc


_NC = None


def _get_nc():
    global _NC
    if _NC is None:
        _NC = build_nc()
    return _NC


def make_in_maps(x, w_attn, b_attn, w_proj):
    x = np.asarray(x, dtype=np.float32)
    w_attn = np.asarray(w_attn, dtype=np.float32)
    b_attn = np.asarray(b_attn, dtype=np.float32)
    w_proj = np.asarray(w_proj, dtype=np.float32)
    in_maps = []
    for core in range(8):
        b, g = divmod(core, 2)
        s = g * CQ
        in_maps.append({
            "x": np.ascontiguousarray(x[b]).astype(np.float16),
            "w_qk": np.ascontiguousarray(
                np.concatenate([w_attn[:, s:s + CQ], w_attn[:, C + s:C + s + CQ]], axis=1)
            ).astype(np.float16),
            "w_v": np.ascontiguousarray(
                w_attn[:, 2 * C + s:2 * C + s + CQ]).astype(np.float16),
            "b_qk": np.concatenate(
                [b_attn[s:s + CQ], b_attn[C + s:C + s + CQ]]
            ).reshape(1, 2 * CQ).astype(np.float32),
            # v bias replicated across partitions: added on DVE during the
            # v copy (dst rows are T-blocks, bias is per-channel)
            "b_vz": np.broadcast_to(
                b_attn[2 * C + s:2 * C + s + CQ].reshape(1, CQ), (128, CQ)
            ).astype(np.float16).copy(),
            # ones rows at partitions 0 AND 64 (K=1 broadcast stationaries
            # for the two 64-partition halves)
            "onesz": np.concatenate([
                np.ones((1, 128), np.float32),
                np.zeros((63, 128), np.float32),
                np.ones((1, 128), np.float32),
                np.zeros((63, 128), np.float32)]).astype(np.float16),
            "w_pj": np.ascontiguousarray(w_proj[s:s + CQ, :]).astype(np.float16),
        })
    return in_maps


def kernel(x, w_attn, b_attn, w_proj, b_proj):
    nc = _get_nc()
    in_maps = make_in_maps(x, w_attn, b_attn, w_proj)
    res = run_bass_kernel_spmd(nc, in_maps, list(range(8)))
    b_proj = np.asarray(b_proj, dtype=np.float32)
    out = np.empty((B, T, C), dtype=np.float32)
    for b in range(B):
        out[b] = res.results[2 * b]["out"] + res.results[2 * b + 1]["out"] + b_proj
    return out


# revision 16
# speedup vs baseline: 1.0529x; 1.0529x over previous
"""Causal self-attention (B=4, T=2048, C=1024, H=16) on 8 NeuronCores.

Sharding: data-parallel over batch (4) x tensor-parallel over heads (2 groups
of 8 heads) = 8 cores. Each core computes QKV for its 8 heads, causal
flash-style attention, and a partial output projection (row-parallel).
Host sums the two partial projections per batch and adds b_proj.

All matmul operands are stored fp16 (the PE multiplies at ~fp22 internally,
so fp16's 11-bit mantissa matches fp32r precision while halving memory and
enabling hardware DMA-transpose + fast weight loads). All accumulation is
fp32 in PSUM.

Per-core device kernel (Bass/Tile):
  phase 1: x^T loaded via hardware DMA-transpose (fp16); q^T,k^T [ch,T] and
           v [T,ch] (65-col blocks with a ones column that makes the PV
           matmul emit softmax denominators) via fp16 matmuls; qk bias fused
           into the PSUM->SBUF copy, v bias fused into the v copy (DVE add).
  phase 2: per (head, 512-wide tq chunk): scores^T = k^T.T @ q^T in PSUM,
           exp on ACT (scale=1/8) -> P^T fp16, causal handled by restricting
           diagonal-block columns + affine_select zero-fill, PV accumulate
           y^T[65,512] where row 64 = softmax denominator l. Normalization:
           r=1/l broadcast to [64,512] with a K=1 PE matmul, applied on DVE.
           Only phase-1 chunk 0 runs up front; chunks 1-3 are interleaved
           into the attention stream on a quota schedule (chunk c+1 finishes
           during attention chunk c) so ACT-bound exp work overlaps PE-bound
           qkv matmuls from the very start.
  phase 3 (interleaved per tq chunk): out = y^T.T @ w_proj, DMA out (fp32).

PSUM budget (8 banks): scores ring "sg" 2x[128,1024] = 4 banks, matmul ring
"mm" (qk/v/proj/R) 2x[128,512] = 2 banks, y ring "psy" 2x[65,512] = 2 banks.
Scores get their own ring so interleaved phase-1/proj matmuls never wait on
the exp to free a PSUM buffer.
"""

from contextlib import nullcontext

import numpy as np

import concourse.bass as bass
import concourse.mybir as mybir
from concourse import bacc
from concourse.tile import TileContext
from concourse.bass_utils import run_bass_kernel_spmd

B, T, C, H, D = 4, 2048, 1024, 16, 64
CQ = 512          # q (or k or v) channels per core = 8 heads * 64
HPC = 8           # heads per core
F32 = mybir.dt.float32
F16 = mybir.dt.float16
Exp = mybir.ActivationFunctionType.Exp
is_ge = mybir.AluOpType.is_ge

TCH = 512         # phase-1 T-chunk
NCH = T // TCH    # 4 chunks
VSTR = HPC * (D + 1)   # 520: v_ext per-T-block stride (8 heads x 65)


def build_nc(loop_n=1):
    """loop_n > 1 wraps the whole kernel in a device-side repeat loop
    (benchmarking only -- output is identical every iteration)."""
    nc = bacc.Bacc("TRN2", target_bir_lowering=False, debug=False, num_devices=8)

    x = nc.dram_tensor("x", [T, C], F16, kind="ExternalInput")
    # m-major blocked: [p, m*1024 + kk*128 + c] so each m-block is one DMA
    w_qk = nc.dram_tensor("w_qk", [128, 64 * 128], F16, kind="ExternalInput")
    w_v = nc.dram_tensor("w_v", [C, CQ], F16, kind="ExternalInput")
    b_qk = nc.dram_tensor("b_qk", [1, 2 * CQ], F32, kind="ExternalInput")
    b_vz = nc.dram_tensor("b_vz", [128, CQ], F16, kind="ExternalInput")
    onesz = nc.dram_tensor("onesz", [128, 128], F16, kind="ExternalInput")
    w_pj = nc.dram_tensor("w_pj", [CQ, C], F16, kind="ExternalInput")
    out = nc.dram_tensor("out", [T, C], F16, kind="ExternalOutput")

    with TileContext(nc) as tc:
        with (
            tc.tile_pool(name="const", bufs=1) as pc,
            tc.tile_pool(name="persist", bufs=1) as pp,
            tc.tile_pool(name="work", bufs=2) as pw,
            tc.tile_pool(name="psum", bufs=2, space="PSUM") as ps,
            tc.For_i(0, loop_n, 1, staggered_reset=True)
            if loop_n > 1 else nullcontext(),
        ):
            # ---- persistent activations ----
            qT = [pp.tile([128, T], F16, name=f"qT{m}") for m in range(4)]
            # per-head k^T, zero-padded to K=128 so the scores matmul streams
            # the full qT tile at full SBUF bandwidth (the zero half
            # multiplies the sibling head's rows away)
            kZ = [pp.tile([128, T], F16, name=f"kZ{i}") for i in range(HPC)]
            for i in range(HPC):
                z0 = 64 * (1 - i % 2)
                nc.vector.memset(kZ[i][z0:z0 + 64, :], 0.0)
            yT = [pp.tile([128, T], F16, name=f"yT{m}") for m in range(4)]
            v_ext = pp.tile([128, (T // 128) * VSTR], F16, name="v_ext")
            v_ones = v_ext[:].rearrange(
                "p (t i d) -> p t i d", i=HPC, d=D + 1
            )[:, :, :, D:D + 1]
            nc.gpsimd.memset(v_ones, 1.0)

            # ---- constants (emission order = DMA issue order; x chunk-0
            # transposes are issued first inside phase1 below) ----
            w_qk_sb = pc.tile([128, 8 * 1024], F16, name="w_qk_sb")
            b_qk2 = pc.tile([128, 8], F32, name="b_qk2")
            b_vz_sb = pc.tile([128, CQ], F16, name="b_vz_sb")
            onesz_sb = pc.tile([128, 128], F16, name="onesz_sb")
            w_v_sb = pc.tile([128, 8 * 512], F16, name="w_v_sb")
            w_pj_sb = pc.tile([128, 4 * 1024], F16, name="w_pj_sb")

            def load_consts_early():
                # m-major: the first qk matmul group only needs m-block 0
                for m in range(8):
                    nc.sync.dma_start(
                        out=w_qk_sb[:, m * 1024:(m + 1) * 1024],
                        in_=w_qk[:, m * 1024:(m + 1) * 1024],
                    )
                    if m == 0:
                        # per-channel qk bias [128, 8] (partition = ch in tile)
                        nc.sync.dma_start(
                            out=b_qk2[:],
                            in_=b_qk[0, :].rearrange("(m p) -> p m", p=128),
                        )

            def load_consts_mid():
                nc.sync.dma_start(out=onesz_sb[:], in_=onesz[:])
                nc.sync.dma_start(out=b_vz_sb[:], in_=b_vz[:])
                for kk in range(8):
                    nc.sync.dma_start(
                        out=w_v_sb[:, kk * 512:(kk + 1) * 512],
                        in_=w_v[kk * 128:(kk + 1) * 128, :],
                    )

            def load_consts_late():
                for kk in range(4):
                    nc.sync.dma_start(
                        out=w_pj_sb[:, kk * 1024:(kk + 1) * 1024],
                        in_=w_pj[kk * 128:(kk + 1) * 128, :],
                    )

            def phase1_chunk_steps(ct):
                """Yield emission closures for one phase-1 chunk, so chunks
                can be interleaved into the attention stream."""
                T0 = ct * TCH
                xT_c = pw.tile([128, 8 * TCH], F16, name="xT_c", tag="xT_c", bufs=2)

                def xload():
                    # x^T tiles straight from DRAM via hardware DMA transpose
                    for kk in range(8):
                        nc.sync.dma_start_transpose(
                            xT_c[:, kk * TCH:(kk + 1) * TCH],
                            x[T0:T0 + TCH, kk * 128:(kk + 1) * 128],
                        )

                def qk1(m):
                    # q,k: out^T layout [ch, T-chunk]; bias fused into copy
                    qk_ps = ps.tile([128, TCH], F32, name="qk_ps", tag="mm", bufs=2)
                    for kk in range(8):
                        nc.tensor.matmul(
                            qk_ps[:],
                            w_qk_sb[:, m * 1024 + kk * 128:m * 1024 + (kk + 1) * 128],
                            xT_c[:, kk * TCH:(kk + 1) * TCH],
                            start=(kk == 0),
                            stop=(kk == 7),
                        )
                    if m < 4:
                        nc.vector.tensor_scalar_add(
                            qT[m][:, T0:T0 + TCH], qk_ps[:], b_qk2[:, m:m + 1]
                        )
                    else:
                        for half in range(2):
                            ih = 2 * (m - 4) + half
                            rows = slice(64 * half, 64 * half + 64)
                            nc.vector.tensor_scalar_add(
                                kZ[ih][rows, T0:T0 + TCH],
                                qk_ps[rows, :],
                                b_qk2[rows, m:m + 1],
                            )

                def vpart(tt):
                    # v: natural layout [T-block, ch], interleaved into v_ext;
                    # bias added on DVE during the copy (b_vz rows replicated)
                    v_ps = ps.tile([128, CQ], F32, name="v_ps", tag="mm", bufs=2)
                    for kk in range(8):
                        nc.tensor.matmul(
                            v_ps[:],
                            xT_c[:, kk * TCH + tt * 128:kk * TCH + (tt + 1) * 128],
                            w_v_sb[:, kk * 512:(kk + 1) * 512],
                            start=(kk == 0),
                            stop=(kk == 7),
                        )
                    tb = ct * (TCH // 128) + tt
                    dst = v_ext[:, tb * VSTR:(tb + 1) * VSTR].rearrange(
                        "p (i d) -> p i d", d=D + 1
                    )[:, :, 0:D]
                    src = v_ps[:].rearrange("p (i d) -> p i d", d=D)
                    bias = b_vz_sb[:].rearrange("p (i d) -> p i d", d=D)
                    nc.vector.tensor_add(dst, src, bias)

                yield xload
                for m in range(8):
                    yield lambda m=m: qk1(m)
                for t0 in range(TCH // 128):
                    yield lambda t0=t0: vpart(t0)

            # PV pipeline state carried ACROSS heads: each entry is one
            # scores+exp group whose PV matmuls haven't been emitted yet.
            # Keeping the diagonal groups of head i pending into head i+1's
            # stream means the in-order PE never waits on the exp/select
            # chain (it runs head i+1's scores matmuls meanwhile). Entries
            # are tagged with a head serial so normalize() can drain exactly
            # the groups of the head it is about to read.
            pending = []  # [(serial, P tile, y_ps, vslice, nblk, items)]
            head_serial = [0]

            def _emit_pv(entry):
                _, P, y_ps, vslice, nblk, items = entry
                for tkb, oc0, pc0, w in items:
                    nc.tensor.matmul(
                        y_ps[:, oc0:oc0 + w],
                        vslice(tkb),
                        P[:, pc0:pc0 + w],
                        start=(tkb == 0),
                        stop=(tkb == nblk - 1),
                        skip_group_check=True,
                    )

            def flush(depth=0):
                while len(pending) > depth:
                    _emit_pv(pending.pop(0))

            def flush_head(serial):
                while pending and pending[0][0] <= serial:
                    _emit_pv(pending.pop(0))

            def attention(c, i, filler=None):
                """Emit scores+exp groups; PV runs 2 groups behind through
                the shared `pending` pipeline. Returns (y_ps, m, p0, c) for
                deferred normalization."""
                m = i // 2
                p0 = 64 * (i % 2)
                nblk = 4 * c + 4
                head_serial[0] += 1
                serial = head_serial[0]
                y_ps = ps.tile([D + 1, 512], F32, name="y_ps", tag="psy", bufs=2)

                def vslice(tkb):
                    return v_ext[
                        :, tkb * VSTR + i * (D + 1):tkb * VSTR + (i + 1) * (D + 1)
                    ]

                def group(items):
                    """One psum tile + one exp over several blocks.
                    items: (tkb, out_col0, p_col0, width, straddler)."""
                    total = items[-1][2] + items[-1][3]
                    s_g = ps.tile([128, 1024], F32, name="s_g", tag="sg", bufs=2)
                    P_g = pw.tile([128, 1024], F16, name="P_g", tag="P_t", bufs=8)
                    for tkb, oc0, pc0, w, _ in items:
                        nc.tensor.matmul(
                            s_g[:, pc0:pc0 + w],
                            kZ[i][:, tkb * 128:(tkb + 1) * 128],
                            qT[m][:, c * 512 + oc0:(c + 1) * 512],
                            start=True,
                            stop=True,
                        )
                    nc.scalar.activation(
                        P_g[:, 0:total], s_g[:, 0:total], Exp, scale=0.125)
                    for tkb, oc0, pc0, w, straddler in items:
                        if straddler:
                            # keep where (piece-local y) >= x
                            nc.gpsimd.affine_select(
                                out=P_g[:, pc0:pc0 + w],
                                in_=P_g[:, pc0:pc0 + w],
                                compare_op=is_ge,
                                fill=0.0,
                                base=0,
                                pattern=[[1, w]],
                                channel_multiplier=-1,
                            )
                    flush(depth=3)
                    pending.append(
                        (serial, P_g, y_ps, vslice, nblk,
                         [it[:4] for it in items]))
                    if filler is not None:
                        filler()

                # full (below-diagonal) blocks in pairs; diagonal straddlers
                # packed j0+j1 and j2+j3 to amortize ACT fixed cost
                for pair in range(2 * c):
                    group([(2 * pair, 0, 0, 512, False),
                           (2 * pair + 1, 0, 512, 512, False)])
                group([(4 * c, 0, 0, 512, True),
                       (4 * c + 1, 128, 512, 384, True)])
                group([(4 * c + 2, 256, 0, 256, True),
                       (4 * c + 3, 384, 256, 128, True)])
                return (y_ps, m, p0, c, serial)

            o_tiles = {}
            r2_tiles = {}

            def normalize_a(pend):
                """Head i+1's slot: drain head i's PVs, stash unnormalized
                y^T into yT (fp16) and 1/l into the pair's r2 row. Frees the
                y_ps PSUM bank; involves no PE work at all."""
                y_ps, m, p0, c, serial = pend
                flush_head(serial)
                with nc.allow_low_precision(reason="fp16 partial y"):
                    nc.vector.tensor_copy(
                        yT[m][p0:p0 + 64, c * 512:(c + 1) * 512], y_ps[0:D, :])
                r2 = r2_tiles.setdefault(
                    m, pw.tile([65, 512], F16, name="r2", tag="r2", bufs=2))
                with nc.allow_low_precision(reason="fp16 reciprocal"):
                    nc.vector.reciprocal(r2[p0:p0 + 1, :], y_ps[D:D + 1, :])

            def normalize_b(m, c):
                """Per head pair: broadcast both heads' 1/l rows across their
                64-partition halves with two K=1 matmuls into one PSUM bank,
                then one in-place [128,512] mul normalizes both heads.
                Emitted >=1 slot after the pair's reciprocals so the PE never
                waits on the DVE."""
                r2 = r2_tiles.pop(m)
                ones1 = onesz_sb[0:1, 0:64]
                R_ps = ps.tile([128, 512], F32, name="R_ps", tag="mm", bufs=2)
                nc.tensor.matmul(
                    R_ps[0:64, :], ones1, r2[0:1, :], start=True, stop=True)
                nc.tensor.matmul(
                    R_ps[64:128, :], onesz_sb[64:65, 0:64], r2[64:65, :],
                    start=True, stop=True)
                R_sb = pw.tile([128, 512], F16, name="R_sb", tag="R_sb", bufs=2)
                with nc.allow_low_precision(reason="fp16 reciprocal"):
                    nc.vector.tensor_copy(R_sb[:], R_ps[:])
                cols = slice(c * 512, (c + 1) * 512)
                with nc.allow_low_precision(reason="fp16 matches PE fp22 input precision"):
                    nc.vector.tensor_mul(yT[m][:, cols], yT[m][:, cols], R_sb[:])

            def proj(mt, kks=(0, 1, 2, 3), finish=True, nns=(0, 1)):
                """Partial-k projection: kks selects which yT k-blocks to
                accumulate this call. finish=False stashes the partial in o_t
                (fp16) to be completed later by a second call -- used for the
                last chunk so most of its projection hides inside the
                ACT-bound end of attention."""
                o_t = o_tiles.setdefault(
                    mt, pw.tile([128, C], F16, name="o_t", tag="o_t", bufs=6))
                for nn in nns:
                    pj_ps = ps.tile([128, 512], F32, name="pj_ps", tag="mm", bufs=2)
                    for kk in kks:
                        nc.tensor.matmul(
                            pj_ps[:],
                            yT[kk][:, mt * 128:(mt + 1) * 128],
                            w_pj_sb[:, kk * 1024 + nn * 512:kk * 1024 + (nn + 1) * 512],
                            start=(kk == kks[0]),
                            stop=(kk == kks[-1]),
                        )
                    dst = o_t[:, nn * 512:(nn + 1) * 512]
                    with nc.allow_low_precision(reason="fp16 partial output"):
                        if kks[0] == 0:
                            nc.vector.tensor_copy(dst, pj_ps[:])
                        else:
                            nc.vector.tensor_add(dst, dst, pj_ps[:])
                if finish:
                    nc.gpsimd.dma_start(
                        out=out[mt * 128:(mt + 1) * 128, :], in_=o_t[:])
                    del o_tiles[mt]

            # ---- emission: x chunk-0 transposes first (the critical DMA),
            # then the weights they meet in the first matmuls; only chunk 0
            # is computed up front. Later chunks' phase-1 and earlier chunks'
            # projections are spread INSIDE the attention stream (one small
            # step after individual score groups) so the in-order PE FIFO
            # always holds non-dependent work when a score group waits on
            # the exp/select chain.
            c0_steps = list(phase1_chunk_steps(0))
            c0_steps[0]()          # xload chunk 0
            load_consts_early()    # w_qk, b_qk2
            load_consts_mid()      # onesz, b_vz, w_v
            for step in c0_steps[1:]:
                step()
            load_consts_late()     # w_pj

            pend = None
            proj_q = []            # proj half-steps (nn granularity)
            proj_quota = {0: 0, 1: 4, 2: 8, 3: 12}   # in halves
            last_c = T // 512 - 1
            for c in range(T // 512):
                p1_items = (list(phase1_chunk_steps(c + 1))
                            if c + 1 < NCH else [])
                quota = min(proj_quota[c], len(proj_q))
                total_groups = HPC * (2 * c + 2)
                p1_deadline = total_groups - 8   # p1 done one head early
                state = {"g": 0, "p1": 0, "pj": 0}

                def filler(p1_items=p1_items, quota=quota,
                           total_groups=total_groups,
                           p1_deadline=p1_deadline, state=state):
                    state["g"] += 1
                    g = state["g"]
                    while (state["p1"] < len(p1_items)
                           and state["p1"] * p1_deadline < len(p1_items) * g):
                        p1_items[state["p1"]]()
                        state["p1"] += 1
                    while (state["pj"] < quota
                           and state["pj"] * total_groups < quota * g):
                        proj_q.pop(0)()
                        state["pj"] += 1

                for i in range(HPC):
                    nxt = attention(c, i, filler)
                    if pend is not None:
                        normalize_a(pend)
                        if pend[2] == 64:   # odd head: its pair is complete
                            normalize_b(pend[1], pend[3])
                    pend = nxt
                    # last chunk: its own projection (k-blocks 0-2, i.e.
                    # heads 0-5, already normalized) fills the ACT-bound end
                    if c == last_c and i >= 6:
                        for mt in (4 * c + 2 * (i - 6), 4 * c + 2 * (i - 6) + 1):
                            proj(mt, kks=(0, 1, 2), finish=False)
                # drain any unconsumed fillers, then close the chunk
                for f in p1_items[state["p1"]:]:
                    f()
                for _ in range(state["pj"], quota):
                    proj_q.pop(0)()
                flush()
                normalize_a(pend)
                normalize_b(pend[1], pend[3])
                pend = None
                if c != last_c:
                    for mt in range(4 * c, 4 * c + 4):
                        proj_q.append(
                            lambda mt=mt: proj(mt, nns=(0,), finish=False))
                        proj_q.append(
                            lambda mt=mt: proj(mt, nns=(1,), finish=True))
            for f in proj_q:
                f()
            for mt in range(4 * last_c, 4 * last_c + 4):
                proj(mt, kks=(3,), finish=True)

    nc.compile()
    return nc


_NC = None


def _get_nc():
    global _NC
    if _NC is None:
        _NC = build_nc()
    return _NC


def make_in_maps(x, w_attn, b_attn, w_proj):
    x = np.asarray(x, dtype=np.float32)
    w_attn = np.asarray(w_attn, dtype=np.float32)
    b_attn = np.asarray(b_attn, dtype=np.float32)
    w_proj = np.asarray(w_proj, dtype=np.float32)
    in_maps = []
    for core in range(8):
        b, g = divmod(core, 2)
        s = g * CQ
        in_maps.append({
            "x": np.ascontiguousarray(x[b]).astype(np.float16),
            # [kk*128+p, m*128+c] -> [p, (m kk c)] m-major blocks
            "w_qk": np.ascontiguousarray(
                np.concatenate([w_attn[:, s:s + CQ], w_attn[:, C + s:C + s + CQ]], axis=1)
                .reshape(8, 128, 8, 128).transpose(1, 2, 0, 3).reshape(128, 8192)
            ).astype(np.float16),
            "w_v": np.ascontiguousarray(
                w_attn[:, 2 * C + s:2 * C + s + CQ]).astype(np.float16),
            "b_qk": np.concatenate(
                [b_attn[s:s + CQ], b_attn[C + s:C + s + CQ]]
            ).reshape(1, 2 * CQ).astype(np.float32),
            # v bias replicated across partitions: added on DVE during the
            # v copy (dst rows are T-blocks, bias is per-channel)
            "b_vz": np.broadcast_to(
                b_attn[2 * C + s:2 * C + s + CQ].reshape(1, CQ), (128, CQ)
            ).astype(np.float16).copy(),
            # ones rows at partitions 0 AND 64 (K=1 broadcast stationaries
            # for the two 64-partition halves)
            "onesz": np.concatenate([
                np.ones((1, 128), np.float32),
                np.zeros((63, 128), np.float32),
                np.ones((1, 128), np.float32),
                np.zeros((63, 128), np.float32)]).astype(np.float16),
            "w_pj": np.ascontiguousarray(w_proj[s:s + CQ, :]).astype(np.float16),
        })
    return in_maps


def kernel(x, w_attn, b_attn, w_proj, b_proj):
    nc = _get_nc()
    in_maps = make_in_maps(x, w_attn, b_attn, w_proj)
    res = run_bass_kernel_spmd(nc, in_maps, list(range(8)))
    b_proj = np.asarray(b_proj, dtype=np.float32)
    out = np.empty((B, T, C), dtype=np.float32)
    for b in range(B):
        out[b] = res.results[2 * b]["out"] + res.results[2 * b + 1]["out"] + b_proj
    return out


# revision 19
# speedup vs baseline: 1.3574x; 1.2892x over previous
"""Causal self-attention (B=4, T=2048, C=1024, H=16) on 8 NeuronCores.

Sharding: data-parallel over batch (4) x tensor-parallel over heads (2 groups
of 8 heads) = 8 cores. Each core computes QKV for its 8 heads, causal
flash-style attention, and a partial output projection (row-parallel).
Host sums the two partial projections per batch and adds b_proj.

All matmul operands are stored fp16 (the PE multiplies at ~fp22 internally,
so fp16's 11-bit mantissa matches fp32r precision while halving memory and
enabling hardware DMA-transpose + fast weight loads). All accumulation is
fp32 in PSUM.

Per-core device kernel (Bass/Tile):
  phase 1: x^T loaded via hardware DMA-transpose (fp16); q^T,k^T [ch,T] and
           v [T,ch] (65-col blocks with a ones column that makes the PV
           matmul emit softmax denominators) via fp16 matmuls; qk bias fused
           into the PSUM->SBUF copy, v bias fused into the v copy (DVE add).
  phase 2: per (head, 512-wide tq chunk): scores^T = k^T.T @ q^T in PSUM,
           exp on ACT (scale=1/8) -> P^T fp16, causal handled by restricting
           diagonal-block columns + affine_select zero-fill, PV accumulate
           y^T[65,512] where row 64 = softmax denominator l. Normalization:
           r=1/l broadcast to [64,512] with a K=1 PE matmul, applied on DVE.
           Only phase-1 chunk 0 runs up front; chunks 1-3 are interleaved
           into the attention stream on a quota schedule (chunk c+1 finishes
           during attention chunk c) so ACT-bound exp work overlaps PE-bound
           qkv matmuls from the very start.
  phase 3 (interleaved per tq chunk): out = y^T.T @ w_proj, DMA out (fp32).

PSUM budget (8 banks): scores ring "sg" 2x[128,1024] = 4 banks, matmul ring
"mm" (qk/v/proj/R) 2x[128,512] = 2 banks, y ring "psy" 2x[65,512] = 2 banks.
Scores get their own ring so interleaved phase-1/proj matmuls never wait on
the exp to free a PSUM buffer.
"""

from contextlib import nullcontext

import numpy as np

import concourse.bass as bass
import concourse.mybir as mybir
from concourse import bacc
from concourse.tile import TileContext
from concourse.bass_utils import run_bass_kernel_spmd

B, T, C, H, D = 4, 2048, 1024, 16, 64
CQ = 512          # q (or k or v) channels per core = 8 heads * 64
HPC = 8           # heads per core
F32 = mybir.dt.float32
F16 = mybir.dt.float16
Exp = mybir.ActivationFunctionType.Exp
is_ge = mybir.AluOpType.is_ge

TCH = 512         # phase-1 T-chunk
NCH = T // TCH    # 4 chunks
VSTR = HPC * (D + 1)   # 520: v_ext per-T-block stride (8 heads x 65)


def build_nc(loop_n=1):
    """loop_n > 1 wraps the whole kernel in a device-side repeat loop
    (benchmarking only -- output is identical every iteration)."""
    nc = bacc.Bacc("TRN2", target_bir_lowering=False, debug=False, num_devices=8)

    x = nc.dram_tensor("x", [T, C], F16, kind="ExternalInput")
    # m-major blocked: [p, m*1024 + kk*128 + c] so each m-block is one DMA
    w_qk = nc.dram_tensor("w_qk", [128, 64 * 128], F16, kind="ExternalInput")
    w_v = nc.dram_tensor("w_v", [C, CQ], F16, kind="ExternalInput")
    b_qk = nc.dram_tensor("b_qk", [1, 2 * CQ], F32, kind="ExternalInput")
    b_vz = nc.dram_tensor("b_vz", [128, CQ], F16, kind="ExternalInput")
    onesz = nc.dram_tensor("onesz", [128, 128], F16, kind="ExternalInput")
    w_pj = nc.dram_tensor("w_pj", [CQ, C], F16, kind="ExternalInput")
    out = nc.dram_tensor("out", [T, C], F16, kind="ExternalOutput")

    with TileContext(nc) as tc:
        with (
            tc.tile_pool(name="const", bufs=1) as pc,
            tc.tile_pool(name="persist", bufs=1) as pp,
            tc.tile_pool(name="work", bufs=2) as pw,
            tc.tile_pool(name="psum", bufs=2, space="PSUM") as ps,
            tc.For_i(0, loop_n, 1, staggered_reset=True)
            if loop_n > 1 else nullcontext(),
        ):
            # ---- persistent activations ----
            qT = [pp.tile([128, T], F16, name=f"qT{m}") for m in range(4)]
            # per-head k^T, zero-padded to K=128 so the scores matmul streams
            # the full qT tile at full SBUF bandwidth (the zero half
            # multiplies the sibling head's rows away)
            kZ = [pp.tile([128, T], F16, name=f"kZ{i}") for i in range(HPC)]
            for i in range(HPC):
                z0 = 64 * (1 - i % 2)
                nc.vector.memset(kZ[i][z0:z0 + 64, :], 0.0)
            yT = [pp.tile([128, T], F16, name=f"yT{m}") for m in range(4)]
            v_ext = pp.tile([128, (T // 128) * VSTR], F16, name="v_ext")
            v_ones = v_ext[:].rearrange(
                "p (t i d) -> p t i d", i=HPC, d=D + 1
            )[:, :, :, D:D + 1]
            nc.gpsimd.memset(v_ones, 1.0)

            # ---- constants (emission order = DMA issue order; x chunk-0
            # transposes are issued first inside phase1 below) ----
            w_qk_sb = pc.tile([128, 8 * 1024], F16, name="w_qk_sb")
            b_qk2 = pc.tile([128, 8], F32, name="b_qk2")
            b_vz_sb = pc.tile([128, CQ], F16, name="b_vz_sb")
            onesz_sb = pc.tile([128, 128], F16, name="onesz_sb")
            w_v_sb = pc.tile([128, 8 * 512], F16, name="w_v_sb")
            w_pj_sb = pc.tile([128, 4 * 1024], F16, name="w_pj_sb")

            def load_consts_early():
                # m-major: the first qk matmul group only needs m-block 0
                for m in range(8):
                    nc.sync.dma_start(
                        out=w_qk_sb[:, m * 1024:(m + 1) * 1024],
                        in_=w_qk[:, m * 1024:(m + 1) * 1024],
                    )
                    if m == 0:
                        # per-channel qk bias [128, 8] (partition = ch in tile)
                        nc.sync.dma_start(
                            out=b_qk2[:],
                            in_=b_qk[0, :].rearrange("(m p) -> p m", p=128),
                        )

            def load_consts_mid():
                nc.sync.dma_start(out=onesz_sb[:], in_=onesz[:])
                nc.sync.dma_start(out=b_vz_sb[:], in_=b_vz[:])
                for kk in range(8):
                    nc.sync.dma_start(
                        out=w_v_sb[:, kk * 512:(kk + 1) * 512],
                        in_=w_v[kk * 128:(kk + 1) * 128, :],
                    )

            def load_consts_late():
                for kk in range(4):
                    nc.sync.dma_start(
                        out=w_pj_sb[:, kk * 1024:(kk + 1) * 1024],
                        in_=w_pj[kk * 128:(kk + 1) * 128, :],
                    )

            def phase1_steps(T0, TL):
                """Return (xload, [compute closures]) for a [T0, T0+TL) span.
                xload is scheduled separately (2 chunks ahead) so the slow HW
                transpose DMA is always fully hidden."""
                xT_c = pw.tile([128, 8 * TL], F16, name="xT_c", tag="xT_c", bufs=3)

                def xload():
                    # x^T tiles straight from DRAM via hardware DMA transpose
                    for kk in range(8):
                        nc.sync.dma_start_transpose(
                            xT_c[:, kk * TL:(kk + 1) * TL],
                            x[T0:T0 + TL, kk * 128:(kk + 1) * 128],
                        )

                def qk1(m):
                    # q,k: out^T layout [ch, T-span]; bias fused into copy
                    qk_ps = ps.tile([128, TL], F32, name="qk_ps", tag="mm", bufs=2)
                    for kk in range(8):
                        nc.tensor.matmul(
                            qk_ps[:],
                            w_qk_sb[:, m * 1024 + kk * 128:m * 1024 + (kk + 1) * 128],
                            xT_c[:, kk * TL:(kk + 1) * TL],
                            start=(kk == 0),
                            stop=(kk == 7),
                        )
                    if m < 4:
                        nc.vector.tensor_scalar_add(
                            qT[m][:, T0:T0 + TL], qk_ps[:], b_qk2[:, m:m + 1]
                        )
                    else:
                        for half in range(2):
                            ih = 2 * (m - 4) + half
                            rows = slice(64 * half, 64 * half + 64)
                            nc.vector.tensor_scalar_add(
                                kZ[ih][rows, T0:T0 + TL],
                                qk_ps[rows, :],
                                b_qk2[rows, m:m + 1],
                            )

                def vpart(tt):
                    # v: natural layout [T-block, ch], interleaved into v_ext;
                    # bias added on DVE during the copy (b_vz rows replicated)
                    v_ps = ps.tile([128, CQ], F32, name="v_ps", tag="mm", bufs=2)
                    for kk in range(8):
                        nc.tensor.matmul(
                            v_ps[:],
                            xT_c[:, kk * TL + tt * 128:kk * TL + (tt + 1) * 128],
                            w_v_sb[:, kk * 512:(kk + 1) * 512],
                            start=(kk == 0),
                            stop=(kk == 7),
                        )
                    tb = T0 // 128 + tt
                    dst = v_ext[:, tb * VSTR:(tb + 1) * VSTR].rearrange(
                        "p (i d) -> p i d", d=D + 1
                    )[:, :, 0:D]
                    src = v_ps[:].rearrange("p (i d) -> p i d", d=D)
                    bias = b_vz_sb[:].rearrange("p (i d) -> p i d", d=D)
                    nc.vector.tensor_add(dst, src, bias)

                steps = [lambda m=m: qk1(m) for m in range(8)]
                steps += [lambda t0=t0: vpart(t0) for t0 in range(TL // 128)]
                return xload, steps

            # PV pipeline state carried ACROSS heads: each entry is one
            # scores+exp group whose PV matmuls haven't been emitted yet.
            # Keeping the diagonal groups of head i pending into head i+1's
            # stream means the in-order PE never waits on the exp/select
            # chain (it runs head i+1's scores matmuls meanwhile). Entries
            # are tagged with a head serial so normalize() can drain exactly
            # the groups of the head it is about to read.
            pending = []  # [(serial, P tile, y_ps, vslice, nblk, items)]
            head_serial = [0]

            def _emit_pv(entry):
                _, P, y_ps, vslice, nblk, items = entry
                for tkb, oc0, pc0, w in items:
                    nc.tensor.matmul(
                        y_ps[:, oc0:oc0 + w],
                        vslice(tkb),
                        P[:, pc0:pc0 + w],
                        start=(tkb == 0),
                        stop=(tkb == nblk - 1),
                        skip_group_check=True,
                    )

            def flush(depth=0):
                while len(pending) > depth:
                    _emit_pv(pending.pop(0))

            def flush_head(serial):
                while pending and pending[0][0] <= serial:
                    _emit_pv(pending.pop(0))

            def attention(c, i, filler=None):
                """Emit scores+exp groups; PV runs 2 groups behind through
                the shared `pending` pipeline. Returns (y_ps, m, p0, c) for
                deferred normalization."""
                m = i // 2
                p0 = 64 * (i % 2)
                nblk = 4 * c + 4
                head_serial[0] += 1
                serial = head_serial[0]
                y_ps = ps.tile([D + 1, 512], F32, name="y_ps", tag="psy", bufs=2)

                def vslice(tkb):
                    return v_ext[
                        :, tkb * VSTR + i * (D + 1):tkb * VSTR + (i + 1) * (D + 1)
                    ]

                def group(items):
                    """One psum tile + one exp over several blocks.
                    items: (tkb, out_col0, p_col0, width, straddler)."""
                    total = items[-1][2] + items[-1][3]
                    s_g = ps.tile([128, 1024], F32, name="s_g", tag="sg", bufs=2)
                    P_g = pw.tile([128, 1024], F16, name="P_g", tag="P_t", bufs=8)
                    for tkb, oc0, pc0, w, _ in items:
                        nc.tensor.matmul(
                            s_g[:, pc0:pc0 + w],
                            kZ[i][:, tkb * 128:(tkb + 1) * 128],
                            qT[m][:, c * 512 + oc0:(c + 1) * 512],
                            start=True,
                            stop=True,
                        )
                    nc.scalar.activation(
                        P_g[:, 0:total], s_g[:, 0:total], Exp, scale=0.125)
                    for tkb, oc0, pc0, w, straddler in items:
                        if straddler:
                            # keep where (piece-local y) >= x
                            nc.gpsimd.affine_select(
                                out=P_g[:, pc0:pc0 + w],
                                in_=P_g[:, pc0:pc0 + w],
                                compare_op=is_ge,
                                fill=0.0,
                                base=0,
                                pattern=[[1, w]],
                                channel_multiplier=-1,
                            )
                    flush(depth=3)
                    pending.append(
                        (serial, P_g, y_ps, vslice, nblk,
                         [it[:4] for it in items]))
                    if filler is not None:
                        filler()

                # full (below-diagonal) blocks in pairs; diagonal straddlers
                # packed j0+j1 and j2+j3 to amortize ACT fixed cost
                for pair in range(2 * c):
                    group([(2 * pair, 0, 0, 512, False),
                           (2 * pair + 1, 0, 512, 512, False)])
                group([(4 * c, 0, 0, 512, True),
                       (4 * c + 1, 128, 512, 384, True)])
                group([(4 * c + 2, 256, 0, 256, True),
                       (4 * c + 3, 384, 256, 128, True)])
                return (y_ps, m, p0, c, serial)

            o_tiles = {}
            r2_tiles = {}

            def normalize_a(pend):
                """Head i+1's slot: drain head i's PVs, stash unnormalized
                y^T into yT (fp16) and 1/l into the pair's r2 row. Frees the
                y_ps PSUM bank; involves no PE work at all."""
                y_ps, m, p0, c, serial = pend
                flush_head(serial)
                with nc.allow_low_precision(reason="fp16 partial y"):
                    nc.vector.tensor_copy(
                        yT[m][p0:p0 + 64, c * 512:(c + 1) * 512], y_ps[0:D, :])
                r2 = r2_tiles.setdefault(
                    m, pw.tile([65, 512], F16, name="r2", tag="r2", bufs=2))
                with nc.allow_low_precision(reason="fp16 reciprocal"):
                    nc.vector.reciprocal(r2[p0:p0 + 1, :], y_ps[D:D + 1, :])

            def normalize_b(m, c):
                """Per head pair: broadcast both heads' 1/l rows across their
                64-partition halves with two K=1 matmuls into one PSUM bank,
                then one in-place [128,512] mul normalizes both heads.
                Emitted >=1 slot after the pair's reciprocals so the PE never
                waits on the DVE."""
                r2 = r2_tiles.pop(m)
                ones1 = onesz_sb[0:1, 0:64]
                R_ps = ps.tile([128, 512], F32, name="R_ps", tag="mm", bufs=2)
                nc.tensor.matmul(
                    R_ps[0:64, :], ones1, r2[0:1, :], start=True, stop=True)
                nc.tensor.matmul(
                    R_ps[64:128, :], onesz_sb[64:65, 0:64], r2[64:65, :],
                    start=True, stop=True)
                R_sb = pw.tile([128, 512], F16, name="R_sb", tag="R_sb", bufs=2)
                with nc.allow_low_precision(reason="fp16 reciprocal"):
                    nc.vector.tensor_copy(R_sb[:], R_ps[:])
                cols = slice(c * 512, (c + 1) * 512)
                with nc.allow_low_precision(reason="fp16 matches PE fp22 input precision"):
                    nc.vector.tensor_mul(yT[m][:, cols], yT[m][:, cols], R_sb[:])

            def proj(mt, kks=(0, 1, 2, 3), finish=True, nns=(0, 1)):
                """Partial-k projection: kks selects which yT k-blocks to
                accumulate this call. finish=False stashes the partial in o_t
                (fp16) to be completed later by a second call -- used for the
                last chunk so most of its projection hides inside the
                ACT-bound end of attention."""
                o_t = o_tiles.setdefault(
                    mt, pw.tile([128, C], F16, name="o_t", tag="o_t", bufs=6))
                for nn in nns:
                    pj_ps = ps.tile([128, 512], F32, name="pj_ps", tag="mm", bufs=2)
                    for kk in kks:
                        nc.tensor.matmul(
                            pj_ps[:],
                            yT[kk][:, mt * 128:(mt + 1) * 128],
                            w_pj_sb[:, kk * 1024 + nn * 512:kk * 1024 + (nn + 1) * 512],
                            start=(kk == kks[0]),
                            stop=(kk == kks[-1]),
                        )
                    dst = o_t[:, nn * 512:(nn + 1) * 512]
                    with nc.allow_low_precision(reason="fp16 partial output"):
                        if kks[0] == 0:
                            nc.vector.tensor_copy(dst, pj_ps[:])
                        else:
                            nc.vector.tensor_add(dst, dst, pj_ps[:])
                if finish:
                    nc.gpsimd.dma_start(
                        out=out[mt * 128:(mt + 1) * 128, :], in_=o_t[:])
                    del o_tiles[mt]

            # ---- emission: x chunk-0 transposes first (the critical DMA),
            # then the weights they meet in the first matmuls; only chunk 0
            # is computed up front. Later chunks' phase-1 and earlier chunks'
            # projections are spread INSIDE the attention stream (one small
            # step after individual score groups) so the in-order PE FIFO
            # always holds non-dependent work when a score group waits on
            # the exp/select chain.
            xl0, st0 = phase1_steps(0, TCH)
            xl0()                  # x chunk-0 transposes (critical DMA)
            load_consts_early()    # w_qk (m-major), b_qk2
            load_consts_mid()      # onesz, b_vz, w_v
            for step in st0:
                step()
            load_consts_late()     # w_pj

            # chunks 1-3: x transposes issue 2 chunks ahead of use
            p1_xl, p1_st = {}, {}
            for ct in range(1, NCH):
                p1_xl[ct], p1_st[ct] = phase1_steps(ct * TCH, TCH)
            p1_xl[1]()

            pend = None
            proj_q = []            # proj half-steps (nn granularity)
            proj_quota = {0: 0, 1: 4, 2: 8, 3: 12}   # in halves
            last_c = T // 512 - 1
            for c in range(T // 512):
                if c + 2 < NCH:
                    p1_xl[c + 2]()
                p1_items = p1_st.get(c + 1, [])
                quota = min(proj_quota[c], len(proj_q))
                total_groups = HPC * (2 * c + 2)
                p1_deadline = total_groups - 8   # p1 done one head early
                state = {"g": 0, "p1": 0, "pj": 0}

                def filler(p1_items=p1_items, quota=quota,
                           total_groups=total_groups,
                           p1_deadline=p1_deadline, state=state):
                    state["g"] += 1
                    g = state["g"]
                    while (state["p1"] < len(p1_items)
                           and state["p1"] * p1_deadline < len(p1_items) * g):
                        p1_items[state["p1"]]()
                        state["p1"] += 1
                    while (state["pj"] < quota
                           and state["pj"] * total_groups < quota * g):
                        proj_q.pop(0)()
                        state["pj"] += 1

                for i in range(HPC):
                    nxt = attention(c, i, filler)
                    if pend is not None:
                        normalize_a(pend)
                        if pend[2] == 64:   # odd head: its pair is complete
                            normalize_b(pend[1], pend[3])
                    pend = nxt
                    # last chunk: its own projection (k-blocks 0-2, i.e.
                    # heads 0-5, already normalized) fills the ACT-bound end
                    if c == last_c and i >= 6:
                        for mt in (4 * c + 2 * (i - 6), 4 * c + 2 * (i - 6) + 1):
                            proj(mt, kks=(0, 1, 2), finish=False)
                # drain any unconsumed fillers, then close the chunk
                for f in p1_items[state["p1"]:]:
                    f()
                for _ in range(state["pj"], quota):
                    proj_q.pop(0)()
                flush()
                normalize_a(pend)
                normalize_b(pend[1], pend[3])
                pend = None
                if c != last_c:
                    for mt in range(4 * c, 4 * c + 4):
                        proj_q.append(
                            lambda mt=mt: proj(mt, nns=(0,), finish=False))
                        proj_q.append(
                            lambda mt=mt: proj(mt, nns=(1,), finish=True))
            for f in proj_q:
                f()
            for mt in range(4 * last_c, 4 * last_c + 4):
                proj(mt, kks=(3,), finish=True)

    nc.compile()
    return nc


_NC = None


def _get_nc():
    global _NC
    if _NC is None:
        _NC = build_nc()
    return _NC


def make_in_maps(x, w_attn, b_attn, w_proj):
    x = np.asarray(x, dtype=np.float32)
    w_attn = np.asarray(w_attn, dtype=np.float32)
    b_attn = np.asarray(b_attn, dtype=np.float32)
    w_proj = np.asarray(w_proj, dtype=np.float32)
    in_maps = []
    for core in range(8):
        b, g = divmod(core, 2)
        s = g * CQ
        in_maps.append({
            "x": np.ascontiguousarray(x[b]).astype(np.float16),
            # [kk*128+p, m*128+c] -> [p, (m kk c)] m-major blocks
            "w_qk": np.ascontiguousarray(
                np.concatenate([w_attn[:, s:s + CQ], w_attn[:, C + s:C + s + CQ]], axis=1)
                .reshape(8, 128, 8, 128).transpose(1, 2, 0, 3).reshape(128, 8192)
            ).astype(np.float16),
            "w_v": np.ascontiguousarray(
                w_attn[:, 2 * C + s:2 * C + s + CQ]).astype(np.float16),
            "b_qk": np.concatenate(
                [b_attn[s:s + CQ], b_attn[C + s:C + s + CQ]]
            ).reshape(1, 2 * CQ).astype(np.float32),
            # v bias replicated across partitions: added on DVE during the
            # v copy (dst rows are T-blocks, bias is per-channel)
            "b_vz": np.broadcast_to(
                b_attn[2 * C + s:2 * C + s + CQ].reshape(1, CQ), (128, CQ)
            ).astype(np.float16).copy(),
            # ones rows at partitions 0 AND 64 (K=1 broadcast stationaries
            # for the two 64-partition halves)
            "onesz": np.concatenate([
                np.ones((1, 128), np.float32),
                np.zeros((63, 128), np.float32),
                np.ones((1, 128), np.float32),
                np.zeros((63, 128), np.float32)]).astype(np.float16),
            "w_pj": np.ascontiguousarray(w_proj[s:s + CQ, :]).astype(np.float16),
        })
    return in_maps


def kernel(x, w_attn, b_attn, w_proj, b_proj):
    nc = _get_nc()
    in_maps = make_in_maps(x, w_attn, b_attn, w_proj)
    res = run_bass_kernel_spmd(nc, in_maps, list(range(8)))
    b_proj = np.asarray(b_proj, dtype=np.float32)
    out = np.empty((B, T, C), dtype=np.float32)
    for b in range(B):
        out[b] = res.results[2 * b]["out"] + res.results[2 * b + 1]["out"] + b_proj
    return out
